# revision 32
# baseline (speedup 1.0000x reference)
"""Trainium2 Bass kernel for Luong bilinear attention.

  out = softmax((q @ w) @ k^T) @ v      q:[B,Lq,Din] k,v:[B,Lk,Dout] w:[Din,Dout]

Sharding: 8 cores = 4 batches x 2 halves of Lq (data-parallel over batch,
sequence-parallel over Lq). k, v are replicated across the 2 cores of a batch.

Per-core layout strategy: scores are computed transposed, sT[k, q], so the
softmax denominator and the attention*V product are both plain matmuls with
k as the contraction (partition) dim:
    wqT[o, q] = w[i,o]^T . qT[i, q]          (PE, fp16)
    sT[k, q]  = kT[o, k]^T . wqT[o, q]       (PE, fp16, f32 PSUM)
    p[k, q]   = exp(sT)                      (ScalarE, f32 -> bf16)
    acc[q, 0:257] = p^T . [v | ones]         (PE; col 256 = softmax denom)
    out[q, o] = acc[:, 0:256] * (1/acc[:, 256])   (DVE, fp16 out)
exp() is applied without max-subtraction: scores ~ N(0, 12.8), |s| < ~70,
exp stays comfortably inside f32/bf16 range, and softmax is shift-invariant.

The kernel is PE-issue-bound in steady state: the tile scheduler
software-pipelines av(qc)'s matmuls into the scores stream (load-bearing —
ScalarE's exp at 16.1us/chunk is slower than the scores matmuls at
13.8us/chunk, so the exp-independent av matmuls are what keep the PE fed),
and in the final schedule the PE runs gapless from warm-up to last matmul
(~116us at fp16, 99.7% of the 2.37GHz fp16 roofline; fp8 fails numerics:
softmax over N(0,12.8) scores amplifies absolute score error, measured
rel err 0.15 vs the 2e-2 gate).

The remaining time is the seams, all DMA-delivery-bound:
  - NEFF preamble ~7us (entry semaphores + sequencer iram load; fixed),
    dma issue from ~6.8us, first data ~8.1us.
  - The input stream flows at only ~0.19-0.3MB/us/core (chip-HBM
    contention: 8 cores stream simultaneously), so the whole head is a
    conveyor: every dma piece is slotted just ahead of its consumption
    time, interleaving kT tiles and v pieces to match the scheduler's
    scores/av interleave. Descriptor generation (~620ns per dma_start,
    serial on the sync sequencer, 1 descriptor per partition per
    contiguous run) is kept off the critical path by marshalling every
    piece partition-major as ONE contiguous DRAM run per partition (v
    bakes its ones/pad columns into DRAM; no memsets).
  - PE warm-up matmuls on a memset dummy (no DMA dependency) cover the
    entry-barrier-to-w+qT0-arrival window sized for the P75 arrival:
    undershooting drops the PE p-state and the next ~6 matmuls run at
    half clock. A 3-matmul bridge after wq(0) similarly covers the
    PSUM->SBUF copy latency before scores(0) can start.
  - PSUM hazards are tile-granular: wq uses one ps tile per output half
    so the ot=1 matmuls don't falsely serialize behind the ot=0 copy.
  - Tail: fp16 output (halves write bytes), last tile's normalize split
    across DVE+ScalarE and its DMA column-split across the sync+scalar
    queues so descriptor gen runs in parallel.
"""

import numpy as np

B, LQ, LK, DIN, DOUT = 4, 4096, 4096, 256, 256
N_CORES = 8
QS = LQ // (N_CORES // B)  # 2048 queries per core
QC = 512                   # q-chunk (matmul free dim)
NQC = QS // QC             # 4 chunks
NKT = LK // 128            # 32 k tiles
KPC = 8                    # kT 512-key pieces
KP = LK // KPC
VN = DOUT + 1              # v plus ones column
VNP = 264                  # padded v row: 264*2B = 528B = 33 SBUF lines, so
                           # every kt row starts 16B-aligned; pad cols are
                           # zero so reading 0:VN is unaffected
VPC = 8                    # v load pieces (fine-grained: see ring comment)
VKT = NKT // VPC           # 4 k-tiles per v piece

_prog_cache: dict = {}


def build_program(repeat: int = 1):
    """Build the (SPMD-identical) per-core Bass program."""
    if repeat in _prog_cache:
        return _prog_cache[repeat]
    from contextlib import ExitStack

    import concourse.bacc as bacc
    import concourse.mybir as mybir
    import concourse.tile as tile

    BF16 = mybir.dt.bfloat16
    FP16 = mybir.dt.float16
    F32 = mybir.dt.float32
    EXP = mybir.ActivationFunctionType.Exp

    nc = bacc.Bacc(
        "TRN2", target_bir_lowering=False, debug=False, num_devices=N_CORES
    )
    # all inputs partition-major: [128, pieces..., free] so each (partition,
    # dma) is a single contiguous DRAM run -> 1 descriptor per partition
    qT0_d = nc.dram_tensor("qT0", [128, 1, 2, QC], FP16, kind="ExternalInput")
    qT123_d = nc.dram_tensor("qT123", [128, 3, 2, QC], FP16, kind="ExternalInput")
    # one dma per 512-key kT piece: the input stream flows at only
    # ~0.19MB/us/core (chip-HBM contention, 8 cores streaming at once), so
    # a merged multi-piece dma posts its semaphore up to ~2us after its
    # first keys are consumable and stalls scores(0) (measured ~0.6-2us);
    # per-piece semaphores track consumption exactly
    kT_ds = [
        nc.dram_tensor(f"kT{i}", [128, 1, 2, KP], FP16, kind="ExternalInput")
        for i in range(KPC)
    ]
    v_d = nc.dram_tensor("v", [VPC, 128, VKT, VNP], BF16, kind="ExternalInput")
    w_d = nc.dram_tensor("w", [128, 2, DOUT], FP16, kind="ExternalInput")
    o_d = nc.dram_tensor("o", [QS, DOUT], FP16, kind="ExternalOutput")

    with tile.TileContext(nc) as tc, ExitStack() as ctx:
        persist = ctx.enter_context(tc.tile_pool(name="persist", bufs=1))
        pexp = ctx.enter_context(tc.tile_pool(name="pexp", bufs=2))
        ps_pool = ctx.enter_context(
            tc.tile_pool(name="ps", bufs=3, space="PSUM")
        )
        po_pool = ctx.enter_context(
            tc.tile_pool(name="po", bufs=2, space="PSUM")
        )
        outp = ctx.enter_context(tc.tile_pool(name="outp", bufs=4))

        NWARM = 17  # sized to cover w+qT0 arrival (10.6-11.9us, jittery):
        # if warmup undershoots the arrival the PE idles and DROPS P-STATE,
        # making the next ~6 matmuls run at half clock (a 1.6us idle cost
        # +2.7us of slow matmuls, measured) — overshoot costs ~0.2us/mm, so
        # cover the P75 arrival, not the typical one.
        for _ in range(repeat):
            # ---- PE warm-up on a memset dummy: no DMA dependency, so the
            # ---- array is busy from the entry barrier and the clock ramps
            # ---- while the first input DMAs are still in flight
            warm = persist.tile([128, 256], FP16, tag="warm")
            nc.vector.memset(warm[:], 1.0)
            wps = ps_pool.tile([128, 2, QC], F32, tag="ps")
            for _i in range(NWARM):
                nc.tensor.matmul(
                    wps[:, 0, 0:256], warm[:, 0:128], warm[:],
                    start=True, stop=True,
                )

            # ---- input loads: one FIFO ring (sync), consumption order.
            w_bf = persist.tile([128, 2, DOUT], FP16, tag="w_bf")
            qT_bf = persist.tile([128, NQC, 2, QC], FP16, tag="qT_bf")
            kT_bf = persist.tile([128, KPC, 2, KP], FP16, tag="kT_bf")
            v_bf = persist.tile([128, NKT, VNP], BF16, tag="v_bf")

            # single sync ring, consumption order. (Two parallel rings were
            # tried — scalar carrying qT0+kT12 — and did NOT speed up the
            # first arrivals: the early window is DMA-engine-wake-bound,
            # not ring-bound, and cross-ring arbitration added occasional
            # mid-stream stalls.)
            def load_v(vh):
                sl = slice(vh * VKT, (vh + 1) * VKT)
                nc.sync.dma_start(v_bf[:, sl, :], v_d.ap()[vh])

            # Ring order interleaves kT and v pieces to match the tile
            # scheduler's software-pipelined consumption: av(0)'s first
            # accumulation matmuls get interleaved into the scores stream
            # from ~17us (1 av-mm per 2-4 score-mms), so v rows are needed
            # nearly as early as kT columns. The stream flows at only
            # ~0.19-0.3MB/us (chip-HBM contention, 8 cores at once), so
            # every piece is slotted just ahead of its consumption time.
            nc.sync.dma_start(w_bf[:], w_d.ap())
            nc.sync.dma_start(qT_bf[:, 0:1], qT0_d.ap())
            for i in range(3):
                nc.sync.dma_start(kT_bf[:, i : i + 1], kT_ds[i].ap())
            load_v(0)
            nc.sync.dma_start(kT_bf[:, 3:4], kT_ds[3].ap())
            load_v(1)
            nc.sync.dma_start(kT_bf[:, 4:5], kT_ds[4].ap())
            load_v(2)
            nc.sync.dma_start(kT_bf[:, 5:6], kT_ds[5].ap())
            nc.sync.dma_start(qT_bf[:, 1:4], qT123_d.ap())
            load_v(3)
            nc.sync.dma_start(kT_bf[:, 6:7], kT_ds[6].ap())
            load_v(4)
            nc.sync.dma_start(kT_bf[:, 7:8], kT_ds[7].ap())
            load_v(5)
            load_v(6)
            load_v(7)

            # ---- wqT[o, q] = w^T . qT, one qc chunk at a time. Each chunk
            # ---- is emitted just before the scores() phase that consumes
            # ---- it so a late qT piece can never block already-runnable
            # ---- scores work in the in-order PE queue.
            wq_bf = persist.tile([128, 2, QS], FP16, tag="wq_bf")

            def wq(qc):
                # one ps tile PER ot half (same tag/shape, so no extra PSUM
                # footprint): with a shared tile the ot=1 matmuls falsely
                # serialize behind the ot=0 copy (~780ns PE stall measured),
                # since PSUM hazards are tracked at tile granularity
                for ot in range(2):
                    ps = ps_pool.tile([128, 2, QC], F32, tag="ps")
                    for it in range(2):
                        nc.tensor.matmul(
                            ps[:, 0, :],
                            w_bf[:, it, ot * 128 : (ot + 1) * 128],
                            qT_bf[:, qc, it, :],
                            start=(it == 0),
                            stop=(it == 1),
                        )
                    # per-half copy: scores(qc)'s first (it=0) matmuls only
                    # wait for the ot=0 half. For qc=0 (head-critical) the
                    # ot=1 copy runs on ScalarE (idle pre-exp; gpsimd can't
                    # read PSUM) so the two copies overlap instead of
                    # serializing on DVE ahead of scores(0)'s it=1 matmuls;
                    # later chunks have 14us of runway, so DVE is fine.
                    if qc == 0 and ot == 1:
                        nc.scalar.activation(
                            wq_bf[:, ot, qc * QC : (qc + 1) * QC],
                            ps[:, 0, :],
                            mybir.ActivationFunctionType.Copy,
                        )
                    else:
                        nc.vector.tensor_copy(
                            wq_bf[:, ot, qc * QC : (qc + 1) * QC], ps[:, 0, :]
                        )

            # ---- main loop: emit scores(qc+1) before AV(qc) so ScalarE's
            # ---- exp always has PE runway to hide behind
            def scores(qc):
                p_all = pexp.tile([128, NKT, QC], BF16, tag="p_all")
                for ktg in range(NKT // 2):
                    ps = ps_pool.tile([128, 2, QC], F32, tag="ps")
                    for j in range(2):
                        kt = ktg * 2 + j
                        kp, lkt = divmod(kt, 4)
                        for it in range(2):
                            nc.tensor.matmul(
                                ps[:, j, :],
                                kT_bf[:, kp, it, lkt * 128 : (lkt + 1) * 128],
                                wq_bf[:, it, qc * QC : (qc + 1) * QC],
                                start=(it == 0),
                                stop=(it == 1),
                            )
                    nc.scalar.activation(
                        p_all[:, ktg * 2 : (ktg + 1) * 2, :], ps[:, :, :], EXP
                    )
                return p_all

            COPY = mybir.ActivationFunctionType.Copy

            def av(qc, p_all):
                for qt in range(QC // 128):
                    po = po_pool.tile([128, VN], F32, tag="po")
                    for kt in range(NKT):
                        nc.tensor.matmul(
                            po[:],
                            p_all[:, kt, qt * 128 : (qt + 1) * 128],
                            v_bf[:, kt, 0:VN],
                            start=(kt == 0),
                            stop=(kt == NKT - 1),
                        )
                    rec = outp.tile([128, 1], F32, tag="rec")
                    nc.vector.reciprocal(rec[:], po[:, DOUT : DOUT + 1])
                    # fp16 out: halves output DMA bytes; o ~ N(0,1), fp16
                    # rounding adds ~1e-4 rel
                    o_sb = outp.tile([128, DOUT], FP16, tag="o_sb")
                    last = qc == NQC - 1 and qt == QC // 128 - 1
                    if last:
                        # final tile is tail-critical: split the normalize
                        # across DVE and the (now idle) ScalarE so the
                        # output DMA issues ~200ns sooner
                        nc.vector.tensor_scalar_mul(
                            o_sb[:, 0:128], po[:, 0:128], rec[:]
                        )
                        nc.scalar.activation(
                            o_sb[:, 128:DOUT], po[:, 128:DOUT], COPY,
                            scale=rec[:],
                        )
                    else:
                        nc.vector.tensor_scalar_mul(
                            o_sb[:], po[:, 0:DOUT], rec[:]
                        )
                    r0 = (qc * (QC // 128) + qt) * 128
                    if last:
                        # column-split the final write across the sync and
                        # scalar queues: descriptor gen (~0.6us each) runs
                        # in parallel and each half issues as soon as its
                        # engine's half of the normalize lands
                        nc.sync.dma_start(
                            o_d.ap()[r0 : r0 + 128, 0:128], o_sb[:, 0:128]
                        )
                        nc.scalar.dma_start(
                            o_d.ap()[r0 : r0 + 128, 128:DOUT],
                            o_sb[:, 128:DOUT],
                        )
                    else:
                        nc.sync.dma_start(o_d.ap()[r0 : r0 + 128, :], o_sb[:])

            wq(0)
            # bridge: keep the PE array busy (and the p-state up) for the
            # ~0.6us the wq(0) PSUM->SBUF copies take before scores(0) can
            # read wq_bf — otherwise the idle drops the clock and the first
            # scores matmuls run at ~half speed
            bps = ps_pool.tile([128, 2, QC], F32, tag="ps")
            for _i in range(3):
                nc.tensor.matmul(
                    bps[:, 0, 0:256], warm[:, 0:128], warm[:],
                    start=True, stop=True,
                )
            # NOTE: the tile scheduler software-pipelines av(qc)'s matmuls
            # into the scores stream regardless of emission order — and
            # that interleave is load-bearing: ScalarE's exp (16.1us/chunk)
            # is SLOWER than the scores matmuls (13.8us/chunk), so without
            # exp-independent av matmuls mixed in, the PE blocks on ScalarE
            # through the ps-pool rotation (deferring av via priority was
            # measured +4.5us). The ring order above feeds the interleave.
            p_prev = scores(0)
            for qc in range(1, NQC):
                wq(qc)
                p_cur = scores(qc)
                av(qc - 1, p_prev)
                p_prev = p_cur
            av(NQC - 1, p_prev)

    nc.compile()
    _prog_cache[repeat] = nc
    return nc


def make_in_maps(q, k, v, w):
    """Shard + marshal full inputs into per-core input maps.

    Marshalling includes the transpose of q/k, the rounding to the kernel's
    compute dtypes (fp16 score path, bf16 values), and the partition-major
    reblocking that makes every (partition, dma) a single contiguous DRAM
    run (one descriptor per partition per dma_start).
    """
    import ml_dtypes

    q = np.asarray(q, dtype=np.float32)
    k = np.asarray(k, dtype=np.float32)
    v = np.asarray(v, dtype=np.float32)

    # w[i, o] -> [p, t, o] with i = t*128 + p
    w16 = (
        np.asarray(w, dtype=np.float32)
        .astype(np.float16)
        .reshape(2, 128, DOUT)
        .transpose(1, 0, 2)
        .copy()
    )
    # k[b][l, o] -> kT[o, l] -> [p, kp, t, kq] with o = t*128 + p, l = kp*KP + kq
    kT = []
    for b in range(B):
        kb = k[b].T.astype(np.float16)            # [DOUT, LK]
        kb = kb.reshape(2, 128, KPC, KP)          # [t, p, kp, kq]
        kT.append(kb.transpose(1, 2, 0, 3).copy())  # [p, kp, t, kq]
    # v[b][l, o] -> [vh, p, ktl, VNP] with l = (vh*VKT + ktl)*128 + p;
    # ones column at o=256 and zero pad 257.. baked into DRAM so the SBUF
    # destination is stride-free (one run per partition) and needs no memset
    vb = []
    for b in range(B):
        x = np.zeros((LK, VNP), dtype=ml_dtypes.bfloat16)
        x[:, 0:DOUT] = v[b].astype(ml_dtypes.bfloat16)
        x[:, DOUT] = 1.0
        x = x.reshape(VPC, VKT, 128, VNP)         # [vh, ktl, p, o]
        vb.append(x.transpose(0, 2, 1, 3).copy())  # [vh, p, ktl, o]

    in_maps = []
    for c in range(N_CORES):
        b, h = divmod(c, N_CORES // B)
        # q[b][l, i] -> qT[i, lq] -> [p, qc, t, ql] with i = t*128+p
        qb = q[b, h * QS : (h + 1) * QS, :].T.astype(np.float16)  # [DIN, QS]
        qb = qb.reshape(2, 128, NQC, QC)          # [t, p, qc, ql]
        qb = qb.transpose(1, 2, 0, 3).copy()      # [p, qc, t, ql]
        m = {
            "qT0": qb[:, 0:1].copy(),
            "qT123": qb[:, 1:4].copy(),
            "v": vb[b],
            "w": w16,
        }
        for i in range(KPC):
            m[f"kT{i}"] = kT[b][:, i : i + 1].copy()
        in_maps.append(m)
    return in_maps


def kernel(q, v, k, w):
    from concourse import bass_utils

    nc = build_program()
    in_maps = make_in_maps(q, k, v, w)
    res = bass_utils.run_bass_kernel_spmd(nc, in_maps, core_ids=list(range(N_CORES)))
    out = np.empty((B, LQ, DOUT), dtype=np.float32)
    for c in range(N_CORES):
        b, h = divmod(c, N_CORES // B)
        out[b, h * QS : (h + 1) * QS, :] = res.results[c]["o"].astype(np.float32)
    return out
